# revision 31
# baseline (speedup 1.0000x reference)
"""GAT (graph attention) layer on 8 TRN2 NeuronCores via Bass/Tile.

Strategy: dst-range sharding — core c owns destination nodes
[c*6250, (c+1)*6250).  Each core:
  Phase 1: computes h = x@W (bf16), per-node attention logits a_s, a_d
           (fused into one matmul via extended weight matrix), writes a
           node table [h | a_s] to DRAM plus an SBUF-resident per-window
           a_d table (bf16).  PSUM->SBUF copies alternate between the
           scalar and vector engines; table writes go out on the scalar
           engine's DMA queue so they overlap the x reads on sync's.
  Phase 2: processes its edges in 49 dst-windows of 128 dst rows.  Per
           window: dma_gather the per-edge [h|a_s] rows (int16 indices,
           src split in two halves of 25000), broadcast a_d to edges
           with a transposed one-hot (host-packed int8, cast to bf16 on
           the scalar engine) via tiny matmuls, score
           e = exp(leaky_relu(a_s+a_d)) (max-subtraction skipped —
           scores are O(1) so exp never overflows; softmax
           normalization is algebraically deferred to a final divide),
           scale h by e in place, and aggregate per dst row with
           one-hot matmuls accumulated in PSUM.
           out = (sum e*h)/(sum e) + bias.
  The two half-gathers per window rotate across 4 SWDGE queues so
  descriptor generation (the gpsimd-serial cost) overlaps the DMA
  drain of neighbouring gathers; the gather landing tiles are triple
  buffered so gathers run ahead of the compute chain.
No collectives needed; host concatenates the 8 dst shards.
"""
import sys

sys.path.insert(0, "/opt/trn_rl_repo")

import os
import numpy as np

DBG_WIN = int(os.environ.get("DBG_WIN", "-1"))
DBG_STAGE = int(os.environ.get("DBG_STAGE", "9"))
DBG_PAD0 = os.environ.get("DBG_PAD0", "1") == "1"  # 0-pads (runtime
# num_idxs_reg trimming of -1 pads crashes the device; keep 0-pads)
N_QUEUES_ENV = int(os.environ.get("N_QUEUES", "4"))

N_NODES = 50000
N_EDGES = 1600000
IN_DIM = 128
OUT_DIM = 64
HEADS = 4
HF = HEADS * OUT_DIM  # 256
NEG_SLOPE = 0.2
N_CORES = 8
D_PER_CORE = N_NODES // N_CORES  # 6250
HALF = N_NODES // 2  # 25000 (int16-safe index range)
WIN = 128  # dst rows per window
N_WIN = (D_PER_CORE + WIN - 1) // WIN  # 49 (last window 106 rows)
TBL_ROW = 384  # bf16 elems per table row: 256 h + 4 a_s bf16 + pad
HALF_PAD = 25088  # 25000 rounded up to 28*896
CH = 896  # phase-1 chunk cols (7 tiles of 128)
N_CHUNK_HALF = HALF_PAD // CH  # 28
N_QUEUES = N_QUEUES_ENV


def _build_edge_shards(src, dst):
    """Partition/sort edges host-side (index manipulation only).

    Returns per-core packed index arrays plus the global tile counts
    (T0, T1) per window half.
    """
    core = dst // D_PER_CORE
    dst_local = dst - core * D_PER_CORE
    win = dst_local >> 7
    dst_rel = dst_local & 127
    half = (src >= HALF).astype(np.int64)

    group = ((core * N_WIN + win) << 1) | half  # 784 groups
    order = np.argsort(group, kind="stable")
    g_sorted = group[order]
    counts = np.bincount(group, minlength=N_CORES * N_WIN * 2)
    offsets = np.zeros_like(counts)
    np.cumsum(counts[:-1], out=offsets[1:])
    seq = np.arange(src.shape[0], dtype=np.int64) - offsets[g_sorted]

    c0 = counts[0::2].reshape(N_CORES, N_WIN)
    c1 = counts[1::2].reshape(N_CORES, N_WIN)
    T0s = ((c0.max(axis=0) + 127) // 128).astype(np.int64)  # per window
    T1s = ((c1.max(axis=0) + 127) // 128).astype(np.int64)
    Ts = T0s + T1s
    nslot_w = Ts * 128
    slot_off = np.zeros(N_WIN, dtype=np.int64)
    np.cumsum(nslot_w[:-1], out=slot_off[1:])
    t_off = np.zeros(N_WIN, dtype=np.int64)
    np.cumsum(Ts[:-1], out=t_off[1:])
    tot_slots = int(nslot_w.sum())
    tot_tiles = int(Ts.sum())

    # slot within window for every edge (ordered: half0 then half1)
    e_core = core[order]
    e_win = win[order]
    e_half = half[order]
    e_src = src[order]
    e_drel = dst_rel[order]
    slot = np.where(e_half == 0, seq, T0s[e_win] * 128 + seq)

    shards = []
    for c in range(N_CORES):
        m = e_core == c
        w = e_win[m]
        s = slot[m]
        srcv = e_src[m] - e_half[m] * HALF  # local to its half
        drel = e_drel[m]

        fill = 0 if DBG_PAD0 else -1
        sd16 = np.full((16, tot_slots // 16), fill, dtype=np.int16)
        s8 = np.zeros((128, tot_tiles * 128), dtype=np.int8)
        stc8 = np.zeros((128, tot_tiles * 128), dtype=np.int8)

        scol = slot_off[w] // 16 + s // 16
        sd16[s % 16, scol] = srcv.astype(np.int16)
        # gather output layout: edge slot s -> partition s%128, col s//128
        # forward one-hot: s8[e(part), tile, j] = (dst_rel of slot == j)
        s8[s % 128, (t_off[w] + s // 128) * 128 + drel] = 1
        # transposed one-hot: stc8[j, tile, e] = (dst_rel of slot == j)
        stc8[drel, (t_off[w] + s // 128) * 128 + s % 128] = 1
        # per-(window, half) valid-index counts for num_idxs_reg
        cnts = np.empty((1, 2 * N_WIN), dtype=np.int32)
        cnts[0, 0::2] = c0[c]
        cnts[0, 1::2] = c1[c]
        shards.append((np.tile(sd16, (8, 1)), s8, stc8, cnts))
    return shards, T0s, T1s


def _build_graph(T0s, T1s):
    from concourse import bacc, bass, mybir, tile

    Ts = [int(a + b) for a, b in zip(T0s, T1s)]
    tot_slots = sum(t * 128 for t in Ts)
    tot_tiles = sum(Ts)
    T_MAX = max(Ts)
    slot_off = [0]
    t_off = [0]
    for t in Ts[:-1]:
        slot_off.append(slot_off[-1] + t * 128)
        t_off.append(t_off[-1] + t)
    f32 = mybir.dt.float32
    bf16 = mybir.dt.bfloat16
    i16 = mybir.dt.int16
    i8 = mybir.dt.int8

    nc = bacc.Bacc(
        "TRN2", target_bir_lowering=False, debug=False,
        num_swdge_queues=N_QUEUES,
    )

    xT = nc.declare_dram_parameter("xT", [IN_DIM, 2 * HALF_PAD], f32, isOutput=False)
    xTo = nc.declare_dram_parameter("xTo", [IN_DIM, N_WIN * 128], f32, isOutput=False)
    w_p = nc.declare_dram_parameter("w", [IN_DIM, HF], f32, isOutput=False)
    att_p = nc.declare_dram_parameter("att", [128, 2 * HF], f32, isOutput=False)
    bias_p = nc.declare_dram_parameter("bias", [128, HF], f32, isOutput=False)
    sd16_p = nc.declare_dram_parameter("sd16", [128, tot_slots // 16], i16, isOutput=False)
    s8_p = nc.declare_dram_parameter("s8", [128, tot_tiles * 128], i8, isOutput=False)
    stc8_p = nc.declare_dram_parameter("stc8", [128, tot_tiles * 128], i8, isOutput=False)
    cnts_p = nc.declare_dram_parameter("cnts", [1, 2 * N_WIN], mybir.dt.int32, isOutput=False)
    out_p = nc.declare_dram_parameter("out", [D_PER_CORE, HF], f32, isOutput=True)

    table1a = nc.dram_tensor("table1a", [HALF_PAD, TBL_ROW], bf16)
    table1b = nc.dram_tensor("table1b", [HALF_PAD, TBL_ROW], bf16)

    with tile.TileContext(nc) as tc:
        with (
            tc.tile_pool(name="const", bufs=1) as cpool,
            tc.tile_pool(name="ph1", bufs=2) as p1,
            tc.tile_pool(name="ph1ps", bufs=4, space="PSUM") as p1ps,
            tc.tile_pool(name="edge3", bufs=3) as pe3,
            tc.tile_pool(name="edge2", bufs=2) as pe2,
            tc.tile_pool(name="agg", bufs=2, space="PSUM") as pps,
            tc.tile_pool(name="adps", bufs=2, space="PSUM") as pad_ps,
        ):
            from concourse import library_config
            nc.gpsimd.load_library(library_config.mlp)
            w_sb = cpool.tile([IN_DIM, HF], f32)
            nc.sync.dma_start(out=w_sb[:], in_=w_p[:, :])
            att_sb = cpool.tile([128, 2 * HF], f32)
            nc.sync.dma_start(out=att_sb[:], in_=att_p[:, :])
            bias_sb = cpool.tile([128, HF], f32)
            nc.sync.dma_start(out=bias_sb[:], in_=bias_p[:, :])
            cnt_sb = cpool.tile([1, 2 * N_WIN], mybir.dt.int32)
            nc.sync.dma_start(out=cnt_sb[:], in_=cnts_p[:, :])
            wext = cpool.tile([IN_DIM, HF + 8], bf16)
            # per-window a_d table [dst_rel(=partition), win*4+h] in bf16
            adsb = cpool.tile([128, N_WIN * 4], bf16)

            if True:
                # Wext = [W | as_mat | ad_mat] in bf16  (264 cols)
                nc.vector.tensor_copy(out=wext[:, :HF], in_=w_sb[:])
                prod = p1.tile([128, HF], f32)
                red = p1.tile([128, 8], f32)
                nc.vector.tensor_tensor(
                    out=prod[:], in0=w_sb[:], in1=att_sb[:, :HF],
                    op=mybir.AluOpType.mult,
                )
                nc.vector.tensor_reduce(
                    out=red[:, 0:4],
                    in_=prod[:].rearrange("p (h f) -> p h f", h=HEADS),
                    axis=mybir.AxisListType.X, op=mybir.AluOpType.add,
                )
                nc.vector.tensor_tensor(
                    out=prod[:], in0=w_sb[:], in1=att_sb[:, HF:],
                    op=mybir.AluOpType.mult,
                )
                nc.vector.tensor_reduce(
                    out=red[:, 4:8],
                    in_=prod[:].rearrange("p (h f) -> p h f", h=HEADS),
                    axis=mybir.AxisListType.X, op=mybir.AluOpType.add,
                )
                nc.vector.tensor_copy(out=wext[:, HF:], in_=red[:])

                # ---- phase 1: node tables [h | a_s] per src-half ----
                for half, tbl in ((0, table1a), (1, table1b)):
                    for ci in range(N_CHUNK_HALF):
                        c0 = half * HALF_PAD + ci * CH
                        xc = p1.tile([IN_DIM, CH], f32, tag="xc")
                        nc.sync.dma_start(out=xc[:], in_=xT[:, c0 : c0 + CH])
                        xcb = p1.tile([IN_DIM, CH], bf16, tag="xcb")
                        nc.scalar.copy(out=xcb[:], in_=xc[:])
                        t1c = p1.tile([128, CH // 128, TBL_ROW], bf16, tag="t1c")
                        for t in range(CH // 128):
                            hp = p1ps.tile([128, HF + 8], f32, tag="hp")
                            nc.tensor.matmul(
                                out=hp[:],
                                lhsT=xcb[:, t * 128 : (t + 1) * 128],
                                rhs=wext[:],
                                start=True, stop=True,
                            )
                            # alternate PSUM->SBUF copies across engines
                            if t % 2 == 0:
                                nc.scalar.copy(
                                    out=t1c[:, t, : HF + 4], in_=hp[:, : HF + 4]
                                )
                            else:
                                nc.vector.tensor_copy(
                                    out=t1c[:, t, : HF + 4], in_=hp[:, : HF + 4]
                                )
                        # table writes on the scalar queue (overlaps reads)
                        nc.scalar.dma_start(
                            out=tbl[ci * CH : (ci + 1) * CH, : HF + 8].rearrange(
                                "(t p) r -> p t r", p=128
                            ),
                            in_=t1c[:, :, : HF + 8],
                        )
                # local a_d table (own dst rows, from xTo) -> SBUF only
                for t in range(N_WIN):
                    xo = p1.tile([IN_DIM, 128], f32, tag="xo")
                    nc.sync.dma_start(out=xo[:], in_=xTo[:, t * 128 : (t + 1) * 128])
                    xob = p1.tile([IN_DIM, 128], bf16, tag="xob")
                    nc.vector.tensor_copy(out=xob[:], in_=xo[:])
                    adp = p1ps.tile([128, 4], f32, tag="hp")
                    nc.tensor.matmul(
                        out=adp[:], lhsT=xob[:], rhs=wext[:, HF + 4 : HF + 8],
                        start=True, stop=True,
                    )
                    nc.vector.tensor_copy(
                        out=adsb[:, t * 4 : (t + 1) * 4], in_=adp[:]
                    )

            if True:
                # one-time init of the gather landing tiles: window-0/1/2
                # pad slots read whatever is in SBUF; it must be finite
                # (not NaN bit patterns) so pad arithmetic can't poison
                # the PSUM accumulation through 0*inf = NaN.
                for _ in range(3):
                    g1i = pe3.tile([128, T_MAX, TBL_ROW], bf16, tag="g1")
                    nc.vector.memset(g1i[:], 0)

                # ---- phase 2: per dst-window edge processing ----
                n_win_run = N_WIN if DBG_WIN < 0 else DBG_WIN
                for wi in range(n_win_run):
                    T0, T1 = int(T0s[wi]), int(T1s[wi])
                    T = T0 + T1
                    NSLOT = T * 128
                    so16 = slot_off[wi] // 16
                    to = t_off[wi]
                    sd = pe3.tile([128, NSLOT // 16], i16, tag="sd")
                    nc.sync.dma_start(
                        out=sd[:],
                        in_=sd16_p[:, so16 : so16 + NSLOT // 16],
                    )
                    si = sd[:, :]
                    sf8 = pe3.tile([128, T, 128], i8, tag="sf8")
                    nc.sync.dma_start(
                        out=sf8[:].rearrange("p t e -> p (t e)"),
                        in_=s8_p[:, to * 128 : (to + T) * 128],
                    )
                    st8 = pe3.tile([128, T, 128], i8, tag="st8")
                    nc.sync.dma_start(
                        out=st8[:].rearrange("p t e -> p (t e)"),
                        in_=stc8_p[:, to * 128 : (to + T) * 128],
                    )

                    if DBG_STAGE < 1:
                        continue
                    g1 = pe3.tile([128, T, TBL_ROW], bf16, tag="g1")
                    if DBG_PAD0:
                        n0reg, n1reg = T0 * 128, T1 * 128
                    else:
                        n0reg = nc.gpsimd.value_load(
                            cnt_sb[0:1, 2 * wi : 2 * wi + 1],
                            min_val=1, max_val=T0 * 128,
                        )
                        n1reg = nc.gpsimd.value_load(
                            cnt_sb[0:1, 2 * wi + 1 : 2 * wi + 2],
                            min_val=1, max_val=T1 * 128,
                        )
                    nc.gpsimd.dma_gather(
                        out_ap=g1[:, :T0, :],
                        in_ap=table1a[:, :],
                        idxs_ap=si[:, : T0 * 8],
                        num_idxs=T0 * 128, num_idxs_reg=n0reg,
                        elem_size=TBL_ROW, single_packet=False,
                        queue_num=(2 * wi) % N_QUEUES,
                    )
                    nc.gpsimd.dma_gather(
                        out_ap=g1[:, T0:T, :],
                        in_ap=table1b[:, :],
                        idxs_ap=si[:, T0 * 8 :],
                        num_idxs=T1 * 128, num_idxs_reg=n1reg,
                        elem_size=TBL_ROW, single_packet=False,
                        queue_num=(2 * wi + 1) % N_QUEUES,
                    )

                    if DBG_STAGE < 2:
                        continue
                    # one-hot S[e, t, j] and transposed STc[j, t, e]:
                    # cast the host 0/1 int8 masks to bf16 on the scalar
                    # engine (keeps the DVE free for the arithmetic)
                    S = pe2.tile([128, T, WIN], bf16, tag="S")
                    nc.scalar.copy(out=S[:], in_=sf8[:])
                    STc = pe2.tile([128, T, 128], bf16, tag="STc")
                    nc.scalar.copy(out=STc[:], in_=st8[:])
                    # per-edge a_d via tiny matmuls: adg[e, t, h]
                    adg = pad_ps.tile([128, T, 4], f32, tag="adg")
                    for t in range(T):
                        nc.tensor.matmul(
                            out=adg[:, t, :],
                            lhsT=STc[:, t, :],
                            rhs=adsb[:, wi * 4 : (wi + 1) * 4],
                            start=True, stop=True,
                        )

                    # scores: z = a_s[src] + a_d[dst]
                    z = pe2.tile([128, T, 4], f32, tag="z")
                    nc.vector.tensor_tensor(
                        out=z[:],
                        in0=g1[:, :, HF : HF + 4],
                        in1=adg[:],
                        op=mybir.AluOpType.add,
                    )
                    z2 = pe2.tile([128, T, 4], f32, tag="z2")
                    nc.vector.tensor_scalar(
                        out=z2[:], in0=z[:], scalar1=NEG_SLOPE, scalar2=None,
                        op0=mybir.AluOpType.mult,
                    )
                    nc.vector.tensor_tensor(
                        out=z2[:], in0=z[:], in1=z2[:], op=mybir.AluOpType.max,
                    )
                    ex = pe2.tile([128, T, 4], f32, tag="ex")
                    nc.scalar.activation(
                        out=ex[:], in_=z2[:], func=mybir.ActivationFunctionType.Exp
                    )

                    if DBG_STAGE < 3:
                        continue
                    # messages in-place in g1: [:, :, :256] = h * ex (per
                    # head), [:, :, 256:260] = ex (a_s consumed by z)
                    nc.vector.tensor_tensor(
                        out=g1[:, :, :HF].rearrange("p t (h f) -> p t h f", h=HEADS),
                        in0=g1[:, :, :HF].rearrange("p t (h f) -> p t h f", h=HEADS),
                        in1=ex[:].rearrange("p t (h o) -> p t h o", o=1).to_broadcast(
                            [128, T, HEADS, OUT_DIM]
                        ),
                        op=mybir.AluOpType.mult,
                    )
                    nc.scalar.copy(out=g1[:, :, HF : HF + 4], in_=ex[:])

                    pa = pps.tile([128, HF + 4], f32, tag="pa")
                    for t in range(T):
                        nc.tensor.matmul(
                            out=pa[:],
                            lhsT=S[:, t, :],
                            rhs=g1[:, t, : HF + 4],
                            start=(t == 0), stop=(t == T - 1),
                        )

                    rec = pe2.tile([128, 4], f32, tag="rec")
                    nc.vector.reciprocal_approx_fast(out=rec[:], in_=pa[:, HF : HF + 4])
                    outw = pe2.tile([128, HF], f32, tag="outw")
                    for h in range(HEADS):
                        nc.vector.tensor_tensor(
                            out=outw[:, h * OUT_DIM : (h + 1) * OUT_DIM],
                            in0=pa[:, h * OUT_DIM : (h + 1) * OUT_DIM],
                            in1=rec[:, h : h + 1].to_broadcast([128, OUT_DIM]),
                            op=mybir.AluOpType.mult,
                        )
                    nc.vector.tensor_tensor(
                        out=outw[:], in0=outw[:], in1=bias_sb[:],
                        op=mybir.AluOpType.add,
                    )
                    r0 = wi * 128
                    rows = min(128, D_PER_CORE - r0)
                    nc.sync.dma_start(out=out_p[r0 : r0 + rows, :], in_=outw[:rows, :])

    nc.compile()
    return nc


LAST_RES = None


def kernel(x, edge_index, W, att_src, att_dst, bias):
    x = np.asarray(x, dtype=np.float32)
    edge_index = np.asarray(edge_index)
    W = np.asarray(W, dtype=np.float32)
    att_src = np.asarray(att_src, dtype=np.float32)
    att_dst = np.asarray(att_dst, dtype=np.float32)
    bias = np.asarray(bias, dtype=np.float32)

    loops = np.arange(N_NODES, dtype=edge_index.dtype)
    src = np.concatenate([edge_index[0], loops]).astype(np.int64)
    dst = np.concatenate([edge_index[1], loops]).astype(np.int64)

    shards, T0s, T1s = _build_edge_shards(src, dst)

    # replicated dense inputs (layout transforms only)
    xT = np.zeros((IN_DIM, 2 * HALF_PAD), dtype=np.float32)
    xT[:, :HALF] = x.T[:, :HALF]
    xT[:, HALF_PAD : HALF_PAD + HALF] = x.T[:, HALF:]
    att_rep = np.zeros((128, 2 * HF), dtype=np.float32)
    att_rep[:, :HF] = np.broadcast_to(att_src.reshape(1, HF), (128, HF))
    att_rep[:, HF:] = np.broadcast_to(att_dst.reshape(1, HF), (128, HF))
    bias_rep = np.broadcast_to(bias.reshape(1, HF), (128, HF)).copy()

    nc = _build_graph(T0s, T1s)

    in_maps = []
    for c in range(N_CORES):
        sd16, s8, stc8, cnts = shards[c]
        xTo = np.zeros((IN_DIM, N_WIN * 128), dtype=np.float32)
        xTo[:, :D_PER_CORE] = x.T[:, c * D_PER_CORE : (c + 1) * D_PER_CORE]
        in_maps.append(
            {
                "xT": xT, "xTo": xTo, "w": W, "att": att_rep,
                "bias": bias_rep, "sd16": sd16,
                "s8": s8, "stc8": stc8, "cnts": cnts,
            }
        )

    from concourse.bass_utils import run_bass_kernel_spmd

    res = run_bass_kernel_spmd(nc, in_maps, core_ids=list(range(N_CORES)))
    global LAST_RES
    LAST_RES = res
    outs = [res.results[c]["out"] for c in range(N_CORES)]
    return np.concatenate(outs, axis=0).astype(np.float32)


# revision 39
# speedup vs baseline: 1.1527x; 1.1527x over previous
"""GAT (graph attention) layer on 8 TRN2 NeuronCores via Bass/Tile.

Strategy: dst-range sharding — core c owns destination nodes
[c*6250, (c+1)*6250).  Each core:
  Phase 1: computes h = x@W (bf16), per-node attention logits a_s, a_d
           (fused into one matmul via extended weight matrix), writes a
           node table [h | a_s] to DRAM plus an SBUF-resident per-window
           a_d table (bf16).  PSUM->SBUF copies alternate between the
           scalar and vector engines; table writes go out on the scalar
           engine's DMA queue so they overlap the x reads on sync's.
  Phase 2: processes its edges in 49 dst-windows of 128 dst rows.  Per
           window: dma_gather the per-edge [h|a_s] rows (int16 indices,
           src split in two halves of 25000), broadcast a_d to edges
           with a transposed one-hot (host-packed int8, cast to bf16 on
           the scalar engine) via tiny matmuls, score
           e = exp(leaky_relu(a_s+a_d)) (max-subtraction skipped —
           scores are O(1) so exp never overflows; softmax
           normalization is algebraically deferred to a final divide),
           scale h by e in place, and aggregate per dst row with
           one-hot matmuls accumulated in PSUM.
           out = (sum e*h)/(sum e) + bias.
  The two half-gathers per window rotate across 4 SWDGE queues so
  descriptor generation (the gpsimd-serial cost) overlaps the DMA
  drain of neighbouring gathers; the gather landing tiles are triple
  buffered so gathers run ahead of the compute chain.
No collectives needed; host concatenates the 8 dst shards.
"""
import sys

sys.path.insert(0, "/opt/trn_rl_repo")

import os
import numpy as np

DBG_WIN = int(os.environ.get("DBG_WIN", "-1"))
DBG_STAGE = int(os.environ.get("DBG_STAGE", "9"))
DBG_PAD0 = os.environ.get("DBG_PAD0", "1") == "1"  # 0-pads (runtime
# num_idxs_reg trimming of -1 pads crashes the device; keep 0-pads)
N_QUEUES_ENV = int(os.environ.get("N_QUEUES", "4"))

N_NODES = 50000
N_EDGES = 1600000
IN_DIM = 128
OUT_DIM = 64
HEADS = 4
HF = HEADS * OUT_DIM  # 256
NEG_SLOPE = 0.2
N_CORES = 8
D_PER_CORE = N_NODES // N_CORES  # 6250
HALF = N_NODES // 2  # 25000 (int16-safe index range)
WIN = 128  # dst rows per window
N_WIN = (D_PER_CORE + WIN - 1) // WIN  # 49 (last window 106 rows)
TBL_ROW = 384  # bf16 elems per table row: 256 h + 4 a_s bf16 + pad
HALF_PAD = 25088  # 25000 rounded up to 14*1792
CH = 1792  # phase-1 chunk cols (14 tiles of 128)
N_CHUNK_HALF = HALF_PAD // CH  # 14
N_QUEUES = N_QUEUES_ENV


def _build_edge_shards(src, dst):
    """Partition/sort edges host-side (index manipulation only).

    Returns per-core packed index arrays plus the global tile counts
    (T0, T1) per window half.
    """
    core = dst // D_PER_CORE
    dst_local = dst - core * D_PER_CORE
    win = dst_local >> 7
    dst_rel = dst_local & 127
    half = (src >= HALF).astype(np.int64)

    group = ((core * N_WIN + win) << 1) | half  # 784 groups
    order = np.argsort(group, kind="stable")
    g_sorted = group[order]
    counts = np.bincount(group, minlength=N_CORES * N_WIN * 2)
    offsets = np.zeros_like(counts)
    np.cumsum(counts[:-1], out=offsets[1:])
    seq = np.arange(src.shape[0], dtype=np.int64) - offsets[g_sorted]

    c0 = counts[0::2].reshape(N_CORES, N_WIN)
    c1 = counts[1::2].reshape(N_CORES, N_WIN)
    T0s = ((c0.max(axis=0) + 127) // 128).astype(np.int64)  # per window
    T1s = ((c1.max(axis=0) + 127) // 128).astype(np.int64)
    Ts = T0s + T1s
    nslot_w = Ts * 128
    slot_off = np.zeros(N_WIN, dtype=np.int64)
    np.cumsum(nslot_w[:-1], out=slot_off[1:])
    t_off = np.zeros(N_WIN, dtype=np.int64)
    np.cumsum(Ts[:-1], out=t_off[1:])
    tot_slots = int(nslot_w.sum())
    tot_tiles = int(Ts.sum())

    # slot within window for every edge (ordered: half0 then half1)
    e_core = core[order]
    e_win = win[order]
    e_half = half[order]
    e_src = src[order]
    e_drel = dst_rel[order]
    slot = np.where(e_half == 0, seq, T0s[e_win] * 128 + seq)

    shards = []
    for c in range(N_CORES):
        m = e_core == c
        w = e_win[m]
        s = slot[m]
        srcv = e_src[m] - e_half[m] * HALF  # local to its half
        drel = e_drel[m]

        fill = 0 if DBG_PAD0 else -1
        sd16 = np.full((16, tot_slots // 16), fill, dtype=np.int16)
        s8 = np.zeros((128, tot_tiles * 128), dtype=np.int8)
        stc8 = np.zeros((128, tot_tiles * 128), dtype=np.int8)

        scol = slot_off[w] // 16 + s // 16
        sd16[s % 16, scol] = srcv.astype(np.int16)
        # gather output layout: edge slot s -> partition s%128, col s//128
        # forward one-hot: s8[e(part), tile, j] = (dst_rel of slot == j)
        s8[s % 128, (t_off[w] + s // 128) * 128 + drel] = 1
        # transposed one-hot: stc8[j, tile, e] = (dst_rel of slot == j)
        stc8[drel, (t_off[w] + s // 128) * 128 + s % 128] = 1
        # per-(window, half) valid-index counts for num_idxs_reg
        cnts = np.empty((1, 2 * N_WIN), dtype=np.int32)
        cnts[0, 0::2] = c0[c]
        cnts[0, 1::2] = c1[c]
        shards.append((np.tile(sd16, (8, 1)), s8, stc8, cnts))
    return shards, T0s, T1s


def _build_graph(T0s, T1s):
    from concourse import bacc, bass, mybir, tile

    Ts = [int(a + b) for a, b in zip(T0s, T1s)]
    tot_slots = sum(t * 128 for t in Ts)
    tot_tiles = sum(Ts)
    T_MAX = max(Ts)
    slot_off = [0]
    t_off = [0]
    for t in Ts[:-1]:
        slot_off.append(slot_off[-1] + t * 128)
        t_off.append(t_off[-1] + t)
    f32 = mybir.dt.float32
    bf16 = mybir.dt.bfloat16
    i16 = mybir.dt.int16
    i8 = mybir.dt.int8

    nc = bacc.Bacc(
        "TRN2", target_bir_lowering=False, debug=False,
        num_swdge_queues=N_QUEUES,
    )

    xT = nc.declare_dram_parameter("xT", [IN_DIM, 2 * HALF_PAD], f32, isOutput=False)
    xTo = nc.declare_dram_parameter("xTo", [IN_DIM, N_WIN * 128], f32, isOutput=False)
    w_p = nc.declare_dram_parameter("w", [IN_DIM, HF], f32, isOutput=False)
    att_p = nc.declare_dram_parameter("att", [128, 2 * HF], f32, isOutput=False)
    bias_p = nc.declare_dram_parameter("bias", [128, HF], f32, isOutput=False)
    sd16_p = nc.declare_dram_parameter("sd16", [128, tot_slots // 16], i16, isOutput=False)
    s8_p = nc.declare_dram_parameter("s8", [128, tot_tiles * 128], i8, isOutput=False)
    stc8_p = nc.declare_dram_parameter("stc8", [128, tot_tiles * 128], i8, isOutput=False)
    cnts_p = nc.declare_dram_parameter("cnts", [1, 2 * N_WIN], mybir.dt.int32, isOutput=False)
    out_p = nc.declare_dram_parameter("out", [D_PER_CORE, HF], f32, isOutput=True)

    table1a = nc.dram_tensor("table1a", [HALF_PAD, TBL_ROW], bf16)
    table1b = nc.dram_tensor("table1b", [HALF_PAD, TBL_ROW], bf16)

    with tile.TileContext(nc) as tc:
        with tc.tile_pool(name="const", bufs=1) as cpool:
            from concourse import library_config
            nc.gpsimd.load_library(library_config.mlp)
            w_sb = cpool.tile([IN_DIM, HF], f32)
            nc.sync.dma_start(out=w_sb[:], in_=w_p[:, :])
            att_sb = cpool.tile([128, 2 * HF], f32)
            nc.sync.dma_start(out=att_sb[:], in_=att_p[:, :])
            bias_sb = cpool.tile([128, HF], f32)
            nc.sync.dma_start(out=bias_sb[:], in_=bias_p[:, :])
            cnt_sb = cpool.tile([1, 2 * N_WIN], mybir.dt.int32)
            nc.sync.dma_start(out=cnt_sb[:], in_=cnts_p[:, :])
            wext = cpool.tile([IN_DIM, HF + 8], bf16)
            # per-window a_d table [dst_rel(=partition), win*4+h] in bf16
            adsb = cpool.tile([128, N_WIN * 4], bf16)

            with (
                tc.tile_pool(name="ph1", bufs=2) as p1,
                tc.tile_pool(name="ph1ps", bufs=4, space="PSUM") as p1ps,
            ):
                # Wext = [W | as_mat | ad_mat] in bf16  (264 cols)
                nc.vector.tensor_copy(out=wext[:, :HF], in_=w_sb[:])
                prod = p1.tile([128, HF], f32)
                red = p1.tile([128, 8], f32)
                nc.vector.tensor_tensor(
                    out=prod[:], in0=w_sb[:], in1=att_sb[:, :HF],
                    op=mybir.AluOpType.mult,
                )
                nc.vector.tensor_reduce(
                    out=red[:, 0:4],
                    in_=prod[:].rearrange("p (h f) -> p h f", h=HEADS),
                    axis=mybir.AxisListType.X, op=mybir.AluOpType.add,
                )
                nc.vector.tensor_tensor(
                    out=prod[:], in0=w_sb[:], in1=att_sb[:, HF:],
                    op=mybir.AluOpType.mult,
                )
                nc.vector.tensor_reduce(
                    out=red[:, 4:8],
                    in_=prod[:].rearrange("p (h f) -> p h f", h=HEADS),
                    axis=mybir.AxisListType.X, op=mybir.AluOpType.add,
                )
                nc.vector.tensor_copy(out=wext[:, HF:], in_=red[:])

                # ---- phase 1: node tables [h | a_s] per src-half ----
                for half, tbl in ((0, table1a), (1, table1b)):
                    for ci in range(N_CHUNK_HALF):
                        c0 = half * HALF_PAD + ci * CH
                        xc = p1.tile([IN_DIM, CH], f32, tag="xc")
                        nc.sync.dma_start(out=xc[:], in_=xT[:, c0 : c0 + CH])
                        xcb = p1.tile([IN_DIM, CH], bf16, tag="xcb")
                        nc.scalar.copy(out=xcb[:], in_=xc[:])
                        t1c = p1.tile([128, CH // 128, TBL_ROW], bf16, tag="t1c")
                        for t in range(CH // 128):
                            hp = p1ps.tile([128, HF + 8], f32, tag="hp")
                            nc.tensor.matmul(
                                out=hp[:],
                                lhsT=xcb[:, t * 128 : (t + 1) * 128],
                                rhs=wext[:],
                                start=True, stop=True,
                            )
                            # alternate PSUM->SBUF copies across engines
                            if t % 2 == 0:
                                nc.scalar.copy(
                                    out=t1c[:, t, : HF + 4], in_=hp[:, : HF + 4]
                                )
                            else:
                                nc.vector.tensor_copy(
                                    out=t1c[:, t, : HF + 4], in_=hp[:, : HF + 4]
                                )
                        # table writes on the scalar queue (overlaps reads)
                        nc.scalar.dma_start(
                            out=tbl[ci * CH : (ci + 1) * CH, : HF + 8].rearrange(
                                "(t p) r -> p t r", p=128
                            ),
                            in_=t1c[:, :, : HF + 8],
                        )
                # local a_d table (own dst rows, from xTo) -> SBUF only
                for t in range(N_WIN):
                    xo = p1.tile([IN_DIM, 128], f32, tag="xo")
                    nc.sync.dma_start(out=xo[:], in_=xTo[:, t * 128 : (t + 1) * 128])
                    xob = p1.tile([IN_DIM, 128], bf16, tag="xob")
                    nc.vector.tensor_copy(out=xob[:], in_=xo[:])
                    adp = p1ps.tile([128, 4], f32, tag="hp")
                    nc.tensor.matmul(
                        out=adp[:], lhsT=xob[:], rhs=wext[:, HF + 4 : HF + 8],
                        start=True, stop=True,
                    )
                    nc.vector.tensor_copy(
                        out=adsb[:, t * 4 : (t + 1) * 4], in_=adp[:]
                    )

            with (
                tc.tile_pool(name="edge3", bufs=3) as pe3,
                tc.tile_pool(name="edge2", bufs=2) as pe2,
                tc.tile_pool(name="agg", bufs=2, space="PSUM") as pps,
                tc.tile_pool(name="adps", bufs=2, space="PSUM") as pad_ps,
            ):
                # one-time init of the gather landing tiles: window-0/1/2
                # pad slots read whatever is in SBUF; it must be finite
                # (not NaN bit patterns) so pad arithmetic can't poison
                # the PSUM accumulation through 0*inf = NaN.
                for _ in range(3):
                    g1i = pe3.tile([128, T_MAX, TBL_ROW], bf16, tag="g1")
                    nc.vector.memset(g1i[:], 0)

                # ---- phase 2: per dst-window edge processing ----
                # The epilogue (reciprocal/normalize/bias/store) of window
                # N-1 is emitted at the START of window N's block: by then
                # its PSUM accumulation has finished, so it never stalls
                # the DVE queue ahead of window N's score computation.
                def epilogue(wi, pa):
                    rec = pe2.tile([128, 4], f32, tag="rec")
                    nc.vector.reciprocal_approx_fast(
                        out=rec[:], in_=pa[:, HF : HF + 4]
                    )
                    outw = pe2.tile([128, HF], f32, tag="outw")
                    for h in range(HEADS):
                        nc.vector.tensor_tensor(
                            out=outw[:, h * OUT_DIM : (h + 1) * OUT_DIM],
                            in0=pa[:, h * OUT_DIM : (h + 1) * OUT_DIM],
                            in1=rec[:, h : h + 1].to_broadcast([128, OUT_DIM]),
                            op=mybir.AluOpType.mult,
                        )
                    nc.vector.tensor_tensor(
                        out=outw[:], in0=outw[:], in1=bias_sb[:],
                        op=mybir.AluOpType.add,
                    )
                    r0 = wi * 128
                    rows = min(128, D_PER_CORE - r0)
                    nc.sync.dma_start(
                        out=out_p[r0 : r0 + rows, :], in_=outw[:rows, :]
                    )

                prev = None
                n_win_run = N_WIN if DBG_WIN < 0 else DBG_WIN
                for wi in range(n_win_run):
                    T0, T1 = int(T0s[wi]), int(T1s[wi])
                    T = T0 + T1
                    NSLOT = T * 128
                    so16 = slot_off[wi] // 16
                    to = t_off[wi]
                    sd = pe3.tile([128, NSLOT // 16], i16, tag="sd")
                    nc.sync.dma_start(
                        out=sd[:],
                        in_=sd16_p[:, so16 : so16 + NSLOT // 16],
                    )
                    si = sd[:, :]
                    sf8 = pe3.tile([128, T, 128], i8, tag="sf8")
                    nc.sync.dma_start(
                        out=sf8[:].rearrange("p t e -> p (t e)"),
                        in_=s8_p[:, to * 128 : (to + T) * 128],
                    )
                    st8 = pe3.tile([128, T, 128], i8, tag="st8")
                    nc.sync.dma_start(
                        out=st8[:].rearrange("p t e -> p (t e)"),
                        in_=stc8_p[:, to * 128 : (to + T) * 128],
                    )

                    if DBG_STAGE < 1:
                        continue
                    g1 = pe3.tile([128, T, TBL_ROW], bf16, tag="g1")
                    if DBG_PAD0:
                        n0reg, n1reg = T0 * 128, T1 * 128
                    else:
                        n0reg = nc.gpsimd.value_load(
                            cnt_sb[0:1, 2 * wi : 2 * wi + 1],
                            min_val=1, max_val=T0 * 128,
                        )
                        n1reg = nc.gpsimd.value_load(
                            cnt_sb[0:1, 2 * wi + 1 : 2 * wi + 2],
                            min_val=1, max_val=T1 * 128,
                        )
                    nc.gpsimd.dma_gather(
                        out_ap=g1[:, :T0, :],
                        in_ap=table1a[:, :],
                        idxs_ap=si[:, : T0 * 8],
                        num_idxs=T0 * 128, num_idxs_reg=n0reg,
                        elem_size=TBL_ROW, single_packet=False,
                        queue_num=(2 * wi) % N_QUEUES,
                    )
                    nc.gpsimd.dma_gather(
                        out_ap=g1[:, T0:T, :],
                        in_ap=table1b[:, :],
                        idxs_ap=si[:, T0 * 8 :],
                        num_idxs=T1 * 128, num_idxs_reg=n1reg,
                        elem_size=TBL_ROW, single_packet=False,
                        queue_num=(2 * wi + 1) % N_QUEUES,
                    )

                    if DBG_STAGE < 2:
                        continue
                    # one-hot S[e, t, j] and transposed STc[j, t, e]:
                    # cast the host 0/1 int8 masks to bf16 on the scalar
                    # engine (keeps the DVE free for the arithmetic)
                    S = pe2.tile([128, T, WIN], bf16, tag="S")
                    nc.scalar.copy(out=S[:], in_=sf8[:])
                    STc = pe2.tile([128, T, 128], bf16, tag="STc")
                    nc.scalar.copy(out=STc[:], in_=st8[:])
                    # per-edge a_d via tiny matmuls: adg[e, t, h]
                    adg = pad_ps.tile([128, T, 4], f32, tag="adg")
                    for t in range(T):
                        nc.tensor.matmul(
                            out=adg[:, t, :],
                            lhsT=STc[:, t, :],
                            rhs=adsb[:, wi * 4 : (wi + 1) * 4],
                            start=True, stop=True,
                        )

                    # epilogue of the previous window (PSUM now complete)
                    if prev is not None:
                        epilogue(*prev)
                        prev = None

                    # scores: z = a_s[src] + a_d[dst]
                    z = pe2.tile([128, T, 4], f32, tag="z")
                    nc.vector.tensor_tensor(
                        out=z[:],
                        in0=g1[:, :, HF : HF + 4],
                        in1=adg[:],
                        op=mybir.AluOpType.add,
                    )
                    z2 = pe2.tile([128, T, 4], f32, tag="z2")
                    nc.vector.tensor_scalar(
                        out=z2[:], in0=z[:], scalar1=NEG_SLOPE, scalar2=None,
                        op0=mybir.AluOpType.mult,
                    )
                    nc.vector.tensor_tensor(
                        out=z2[:], in0=z[:], in1=z2[:], op=mybir.AluOpType.max,
                    )
                    ex = pe2.tile([128, T, 4], f32, tag="ex")
                    nc.scalar.activation(
                        out=ex[:], in_=z2[:], func=mybir.ActivationFunctionType.Exp
                    )

                    if DBG_STAGE < 3:
                        continue
                    # messages in-place in g1: [:, :, :256] = h * ex (per
                    # head), [:, :, 256:260] = ex (a_s consumed by z)
                    nc.vector.tensor_tensor(
                        out=g1[:, :, :HF].rearrange("p t (h f) -> p t h f", h=HEADS),
                        in0=g1[:, :, :HF].rearrange("p t (h f) -> p t h f", h=HEADS),
                        in1=ex[:].rearrange("p t (h o) -> p t h o", o=1).to_broadcast(
                            [128, T, HEADS, OUT_DIM]
                        ),
                        op=mybir.AluOpType.mult,
                    )
                    nc.scalar.copy(out=g1[:, :, HF : HF + 4], in_=ex[:])

                    pa = pps.tile([128, HF + 4], f32, tag="pa")
                    for t in range(T):
                        nc.tensor.matmul(
                            out=pa[:],
                            lhsT=S[:, t, :],
                            rhs=g1[:, t, : HF + 4],
                            start=(t == 0), stop=(t == T - 1),
                        )
                    prev = (wi, pa)

                if prev is not None:
                    epilogue(*prev)

    nc.compile()
    return nc


LAST_RES = None


def kernel(x, edge_index, W, att_src, att_dst, bias):
    x = np.asarray(x, dtype=np.float32)
    edge_index = np.asarray(edge_index)
    W = np.asarray(W, dtype=np.float32)
    att_src = np.asarray(att_src, dtype=np.float32)
    att_dst = np.asarray(att_dst, dtype=np.float32)
    bias = np.asarray(bias, dtype=np.float32)

    loops = np.arange(N_NODES, dtype=edge_index.dtype)
    src = np.concatenate([edge_index[0], loops]).astype(np.int64)
    dst = np.concatenate([edge_index[1], loops]).astype(np.int64)

    shards, T0s, T1s = _build_edge_shards(src, dst)

    # replicated dense inputs (layout transforms only)
    xT = np.zeros((IN_DIM, 2 * HALF_PAD), dtype=np.float32)
    xT[:, :HALF] = x.T[:, :HALF]
    xT[:, HALF_PAD : HALF_PAD + HALF] = x.T[:, HALF:]
    att_rep = np.zeros((128, 2 * HF), dtype=np.float32)
    att_rep[:, :HF] = np.broadcast_to(att_src.reshape(1, HF), (128, HF))
    att_rep[:, HF:] = np.broadcast_to(att_dst.reshape(1, HF), (128, HF))
    bias_rep = np.broadcast_to(bias.reshape(1, HF), (128, HF)).copy()

    nc = _build_graph(T0s, T1s)

    in_maps = []
    for c in range(N_CORES):
        sd16, s8, stc8, cnts = shards[c]
        xTo = np.zeros((IN_DIM, N_WIN * 128), dtype=np.float32)
        xTo[:, :D_PER_CORE] = x.T[:, c * D_PER_CORE : (c + 1) * D_PER_CORE]
        in_maps.append(
            {
                "xT": xT, "xTo": xTo, "w": W, "att": att_rep,
                "bias": bias_rep, "sd16": sd16,
                "s8": s8, "stc8": stc8, "cnts": cnts,
            }
        )

    from concourse.bass_utils import run_bass_kernel_spmd

    res = run_bass_kernel_spmd(nc, in_maps, core_ids=list(range(N_CORES)))
    global LAST_RES
    LAST_RES = res
    outs = [res.results[c]["out"] for c in range(N_CORES)]
    return np.concatenate(outs, axis=0).astype(np.float32)
